# revision 21
# baseline (speedup 1.0000x reference)
"""Mamba-style block (LN -> softplus -> SSM -> LN -> MLP) on 8 TRN2 NeuronCores.

Sharding: data-parallel over (batch, L-half) -> 8 shards of 512 tokens each.
The SSM scan is sequential in L, so each L-half shard carries a 64-token halo
for scan-state warmup (per-step state decay exp(-delta*|A|) with delta~0.5-0.9
and |A|>=1 makes 64 steps more than enough to reach fp32-exact state).  The
first L-half gets a zero mask over the halo so nothing is injected before t=0.

Layout is feature-major ([D | t]) end to end; matmuls are arranged so no
on-chip transposes are needed until the final residual (one PE transpose).

The scan runs as one tensor_tensor_scan per (d-tile, n-group) over the
flattened (n, t) free axis; segment-boundary contamination between n's decays
inside the halo region exactly like the L-split warmup.
"""

import json as _json
import types
from contextlib import ExitStack

import numpy as np
import ml_dtypes

import concourse.bass as bass
import concourse.tile as tile
from concourse import mybir
from concourse.bass_utils import run_bass_kernel_spmd
from concourse.masks import make_identity


def _split_excess_waits(jmod, maxw=1):
    """The walrus build in this toolchain rejects instructions carrying more
    than a couple of semaphore waits ("Too many sync wait commands").  Tile
    emits instructions with up to ~12 waits.  Post-process the serialized BIR:
    move excess waits onto same-engine NoOps inserted just before the
    instruction (engine program order preserves the blocking semantics)."""
    k = 0
    for fn in jmod["functions"]:
        for blk in fn["blocks"]:
            out = []
            for ins in blk["instructions"]:
                si = ins.get("sync_info")
                waits = (si or {}).get("on_wait") or []
                if len(waits) > maxw:
                    extra, keep = waits[:-maxw], waits[-maxw:]
                    for i in range(0, len(extra), maxw):
                        k += 1
                        out.append({
                            "debug": ins.get("debug", 0),
                            "engine": ins["engine"],
                            "ins": [], "outs": [],
                            "name": f"NW-{k}",
                            "opcode": "NoOp",
                            "sync_info": {"on_wait": extra[i:i + maxw],
                                          "on_update": []},
                        })
                    si["on_wait"] = keep
                out.append(ins)
            blk["instructions"] = out
    return jmod


def _patched_to_json_bytes(self):
    j = _json.loads(mybir.module_to_json_bytes(self.m))
    _split_excess_waits(j)
    return _json.dumps(j).encode()

B, L, D, N, R = 4, 1024, 1024, 16, 64
HID = 4 * D
P = 128
NCORES = 8
TOWN = 512          # owned tokens per core
HALO = 64           # scan warmup tokens
T = TOWN + HALO     # 576
DT = D // P         # 8 d-tiles
HT = HID // P       # 32 hidden tiles
NG = 4              # n groups
NPG = N // NG       # 4 n per group
CH = 288            # free-dim chunk for f32 matmuls over T (576 = 2*288)
TS = TOWN // P      # 4 token tiles (phase D)

F32 = mybir.dt.float32
BF16 = mybir.dt.bfloat16
AX = mybir.AluOpType
AF = mybir.ActivationFunctionType


def _bcast_dram(row_ap, parts=P):
    """AP that replicates a DRAM row across `parts` dest partitions (DMA)."""
    return bass.AP(
        tensor=row_ap.tensor,
        offset=row_ap.offset,
        ap=[[0, parts]] + [list(d) for d in row_ap.ap],
    )


def _bcast_src(row_ap, parts=P):
    """AP that replicates a [1, f...] sbuf row across `parts` partitions (DMA)."""
    return bass.AP(
        tensor=row_ap.tensor,
        offset=row_ap.offset,
        ap=[[0, parts]] + [list(d) for d in row_ap.ap[1:]],
    )


def build_bass():
    nc = bass.Bass()

    x_fm = nc.dram_tensor("x_fm", [D, T], F32, kind="ExternalInput")
    mask_d = nc.dram_tensor("mask", [P, T], F32, kind="ExternalInput")
    wdbcT = nc.dram_tensor("wdbcT", [D, P], F32, kind="ExternalInput")
    wdtT = nc.dram_tensor("wdtT", [R, D], F32, kind="ExternalInput")
    bdt_r = nc.dram_tensor("bdt_r", [P, DT], F32, kind="ExternalInput")
    aneg_r = nc.dram_tensor("aneg_r", [P, DT, N], F32, kind="ExternalInput")
    dp1_r = nc.dram_tensor("dp1_r", [P, DT], F32, kind="ExternalInput")
    w1_r = nc.dram_tensor("w1_r", [P, DT], F32, kind="ExternalInput")
    w2_r = nc.dram_tensor("w2_r", [P, DT], F32, kind="ExternalInput")
    # wfc_t [HT, P, DT, P]: wfc_t[ht, p, dt, c] = W_fc[ht*P+c, dt*P+p]
    wfc_t = nc.dram_tensor("wfc_t", [HT, P, DT, P], BF16, kind="ExternalInput")
    wprojT = nc.dram_tensor("wprojT", [HID, D], BF16, kind="ExternalInput")
    out_d = nc.dram_tensor("out", [TOWN, D], F32, kind="ExternalOutput")

    with tile.TileContext(nc) as tc, ExitStack() as ctx:
        # ------------- persistent pools -------------
        consts = ctx.enter_context(tc.tile_pool(name="consts", bufs=1))
        stat = ctx.enter_context(tc.tile_pool(name="stat", bufs=7))
        hblkp = ctx.enter_context(tc.tile_pool(name="hblk", bufs=1))

        # ------------- constants -------------
        ones1 = consts.tile([P, 1], F32)          # column of ones (stat lhsT)
        nc.vector.memset(ones1, 1.0)
        eps_sb = consts.tile([P, 1], F32)
        nc.vector.memset(eps_sb, 1e-5)
        ident = consts.tile([P, P], F32)
        make_identity(nc, ident)

        mask_sb = consts.tile([P, T], F32)
        nc.sync.dma_start(mask_sb, mask_d[:, :])
        bdt_sb = consts.tile([P, DT], F32)
        nc.sync.dma_start(bdt_sb, bdt_r[:, :])
        aneg_sb = consts.tile([P, DT, N], F32)
        nc.sync.dma_start(aneg_sb, aneg_r[:, :, :])
        dp1_sb = consts.tile([P, DT], F32)
        nc.sync.dma_start(dp1_sb, dp1_r[:, :])
        w1_sb = consts.tile([P, DT], F32)
        nc.sync.dma_start(w1_sb, w1_r[:, :])
        w2_sb = consts.tile([P, DT], F32)
        nc.sync.dma_start(w2_sb, w2_r[:, :])
        wdbc_sb = consts.tile([P, DT, P], F32)
        nc.sync.dma_start(wdbc_sb, wdbcT.rearrange("(o p) e -> p o e", p=P))
        wdt_sb = consts.tile([R, D], F32)
        nc.sync.dma_start(wdt_sb, wdtT[:, :])

        def layernorm_stats(tiles, tag, bc_dst_pool, sq_pool):
            """Feature-major LN stats over partitions via PE ones-matmul.
            tiles: DT x [128, W] f32.  Returns ([128,W] mu_bc, rstd_bc)."""
            W = tiles[0].shape[-1]
            nch = (W + CH - 1) // CH
            chs = [slice(c * CH, min((c + 1) * CH, W)) for c in range(nch)]
            mu = stat.tile([1, W], F32, tag="st")
            msq = stat.tile([1, W], F32, tag="st")
            with tc.tile_pool(name=f"psum_stat{tag}", bufs=1,
                              space="PSUM") as psum_stat:
                ps_s = [psum_stat.tile([1, CH], F32, name=f"ps_s{tag}{c}")
                        for c in range(nch)]
                ps_q = [psum_stat.tile([1, CH], F32, name=f"ps_q{tag}{c}")
                        for c in range(nch)]
                for it, xt in enumerate(tiles):
                    for c, sl in enumerate(chs):
                        w = sl.stop - sl.start
                        xq = sq_pool.tile([P, CH], F32, tag="xsq")
                        nc.vector.tensor_mul(xq[:, :w], xt[:, sl], xt[:, sl])
                        nc.tensor.matmul(ps_s[c][:, :w], ones1, xt[:, sl],
                                         start=(it == 0), stop=(it == DT - 1))
                        nc.tensor.matmul(ps_q[c][:, :w], ones1, xq[:, :w],
                                         start=(it == 0), stop=(it == DT - 1))
                for c, sl in enumerate(chs):
                    w = sl.stop - sl.start
                    nc.scalar.mul(mu[:, sl], ps_s[c][:, :w], 1.0 / D)
                    nc.scalar.mul(msq[:, sl], ps_q[c][:, :w], 1.0 / D)
            var = stat.tile([1, W], F32, tag="st")
            nc.vector.tensor_mul(var, mu, mu)
            nc.vector.tensor_sub(var, msq, var)
            # r0 ~= 1/sqrt(var+eps) = exp(-0.5*ln(var+eps)), Newton-refined
            sq = stat.tile([1, W], F32, tag="st")
            nc.scalar.activation(sq, var, AF.Ln, bias=eps_sb[0:1])
            r0 = stat.tile([1, W], F32, tag="st")
            nc.scalar.activation(r0, sq, AF.Exp, scale=-0.5)
            nc.vector.tensor_scalar_add(var, var, 1e-5)
            # one Newton step: r = r0*(1.5 - 0.5*var*r0^2)
            t1 = stat.tile([1, W], F32, tag="st")
            nc.vector.tensor_mul(t1, r0, r0)
            nc.vector.tensor_mul(t1, t1, var)
            nc.vector.tensor_scalar(t1, t1, -0.5, 1.5, AX.mult, AX.add)
            nc.vector.tensor_mul(r0, r0, t1)
            mu_bc = bc_dst_pool.tile([P, W], F32, name=f"mu_bc{tag}")
            rstd_bc = bc_dst_pool.tile([P, W], F32, name=f"rstd_bc{tag}")
            with tc.tile_pool(name=f"dram_st{tag}", bufs=1,
                              space="DRAM") as dramp:
                st_dram = dramp.tile([2, W], F32)
                nc.sync.dma_start(st_dram[0:1], mu)
                nc.sync.dma_start(st_dram[1:2], r0)
                nc.sync.dma_start(mu_bc, _bcast_dram(st_dram[0, :]))
                nc.sync.dma_start(rstd_bc, _bcast_dram(st_dram[1, :]))
            return mu_bc, rstd_bc

        hblk = [hblkp.tile([P, T], F32, name=f"hblk_{dt}") for dt in range(DT)]

        # ================= phases A-C (scoped pools) =================
        with tc.tile_pool(name="xh1", bufs=1) as xh1, \
             tc.tile_pool(name="xsq", bufs=2) as xsq, \
             tc.tile_pool(name="sb_small", bufs=1) as sb_small, \
             tc.tile_pool(name="bcA", bufs=1) as bcA, \
             tc.tile_pool(name="deltap", bufs=3) as deltap, \
             tc.tile_pool(name="dxp", bufs=3) as dxp:

            # ---- phase A: h1 = mask*softplus(LN1(x)) ----
            xt = []
            for dt in range(DT):
                t = xh1.tile([P, T], F32, name=f"x_{dt}")
                nc.sync.dma_start(t, x_fm[dt * P:(dt + 1) * P, :])
                xt.append(t)
            mu_bc, rstd_bc = layernorm_stats(xt, "1", bcA, xsq)
            h1 = xt  # normalized in place
            for dt in range(DT):
                z = xt[dt]
                nc.vector.tensor_sub(z, z, mu_bc)
                nc.vector.tensor_mul(z, z, rstd_bc)
                nc.scalar.activation(z, z, AF.Exp,
                                     scale=w1_sb[:, dt:dt + 1])
                nc.scalar.activation(z, z, AF.Ln, bias=ones1[:, 0:1])
                nc.vector.tensor_mul(z, z, mask_sb)

            # ---- phase B: dbc, delta, dx, B/C ----
            delta, dxb = [], []
            with tc.tile_pool(name="psum_mm", bufs=2, space="PSUM") as psum_mm:
                ps_dbc = [psum_mm.tile([P, CH], F32, name=f"dbc{c}", bufs=1)
                          for c in range(2)]
                for dt in range(DT):
                    for c in range(2):
                        nc.tensor.matmul(ps_dbc[c], wdbc_sb[:, dt, :],
                                         h1[dt][:, c * CH:(c + 1) * CH],
                                         start=(dt == 0), stop=(dt == DT - 1))
                dlr = sb_small.tile([R, T], F32, name="dlr")
                b_sm = sb_small.tile([N, T], BF16, name="b_sm")
                c_sm = sb_small.tile([N, T], BF16, name="c_sm")
                for c in range(2):
                    sl = slice(c * CH, (c + 1) * CH)
                    nc.scalar.copy(dlr[:, sl], ps_dbc[c][0:R, :])
                    nc.vector.tensor_copy(b_sm[:, sl], ps_dbc[c][64:64 + N, :])
                    nc.vector.tensor_copy(c_sm[:, sl], ps_dbc[c][96:96 + N, :])

                b_bc = bcA.tile([P, N, T], BF16, name="b_bc")
                c_bc = bcA.tile([P, N, T], BF16, name="c_bc")
                with tc.tile_pool(name="dram_bc", bufs=1,
                                  space="DRAM") as dramp:
                    bc_dram = dramp.tile([2, N, T], BF16)
                    nc.sync.dma_start(bc_dram[0], b_sm)
                    nc.sync.dma_start(bc_dram[1], c_sm)
                    for n in range(N):
                        nc.sync.dma_start(b_bc[:, n, :],
                                          _bcast_dram(bc_dram[0, n, :]))
                        nc.sync.dma_start(c_bc[:, n, :],
                                          _bcast_dram(bc_dram[1, n, :]))

                for dt in range(DT):
                    d_t = deltap.tile([P, T], F32, tag="delta")
                    for c in range(2):
                        ps = psum_mm.tile([P, CH], F32, tag="dltps")
                        nc.tensor.matmul(ps, wdt_sb[:, dt * P:(dt + 1) * P],
                                         dlr[:, c * CH:(c + 1) * CH],
                                         start=True, stop=True)
                        nc.scalar.activation(d_t[:, c * CH:(c + 1) * CH], ps,
                                             AF.Exp,
                                             bias=bdt_sb[:, dt:dt + 1])
                        nc.scalar.activation(d_t[:, c * CH:(c + 1) * CH],
                                             d_t[:, c * CH:(c + 1) * CH],
                                             AF.Ln, bias=ones1[:, 0:1])
                    dx_t = dxp.tile([P, T], BF16, tag="dx")
                    nc.vector.tensor_mul(dx_t, d_t, h1[dt])
                    delta.append(d_t)
                    dxb.append(dx_t)

            # ---- phase C: scan volume ----
            with tc.tile_pool(name="dA", bufs=2) as dAp, \
                 tc.tile_pool(name="dBx", bufs=2) as dBxp, \
                 tc.tile_pool(name="hsc", bufs=2) as hp, \
                 tc.tile_pool(name="yssm", bufs=3) as yp:
                for dt in range(DT):
                    yacc = None
                    for g in range(NG):
                        dA = dAp.tile([P, NPG, T], F32, tag="dA")
                        dBx = dBxp.tile([P, NPG, T], BF16, tag="dBx")
                        hsc = hp.tile([P, NPG, T], BF16, tag="hsc")
                        for j in range(NPG):
                            n = g * NPG + j
                            nc.scalar.activation(dA[:, j, :], delta[dt],
                                                 AF.Exp,
                                                 scale=aneg_sb[:, dt, n:n + 1])
                            nc.vector.tensor_mul(dBx[:, j, :], dxb[dt],
                                                 b_bc[:, n, :])
                        nc.vector.tensor_tensor_scan(
                            hsc.rearrange("p a b -> p (a b)"),
                            dA.rearrange("p a b -> p (a b)"),
                            dBx.rearrange("p a b -> p (a b)"),
                            0.0, AX.mult, AX.add)
                        # prod (in place over dBx), then reduce over n
                        for j in range(NPG):
                            n = g * NPG + j
                            nc.vector.tensor_mul(dBx[:, j, :], hsc[:, j, :],
                                                 c_bc[:, n, :])
                        nc.vector.tensor_add(dBx[:, 0, :], dBx[:, 0, :],
                                             dBx[:, 2, :])
                        nc.vector.tensor_add(dBx[:, 1, :], dBx[:, 1, :],
                                             dBx[:, 3, :])
                        if yacc is None:
                            yacc = yp.tile([P, T], BF16, tag="yh")
                            nc.vector.tensor_add(yacc, dBx[:, 0, :],
                                                 dBx[:, 1, :])
                        else:
                            nc.vector.tensor_add(dBx[:, 0, :], dBx[:, 0, :],
                                                 dBx[:, 1, :])
                            nc.vector.tensor_add(yacc, yacc, dBx[:, 0, :])
                    # hblk = h1*(1+Dp) + y
                    nc.vector.scalar_tensor_tensor(hblk[dt], h1[dt],
                                                   dp1_sb[:, dt:dt + 1],
                                                   yacc, AX.mult, AX.add)

        # ================= phase D: LN2 + MLP (owned tokens) =================
        hown = [hb[:, HALO:] for hb in hblk]          # [128, 512] views
        with tc.tile_pool(name="bcD", bufs=1) as bcD, \
             tc.tile_pool(name="sqD", bufs=2) as sqD, \
             tc.tile_pool(name="h2", bufs=1) as h2p, \
             tc.tile_pool(name="wfc", bufs=3) as wfcp, \
             tc.tile_pool(name="ghid", bufs=1) as ghidp, \
             tc.tile_pool(name="wpr", bufs=3) as wprp, \
             tc.tile_pool(name="hT", bufs=1) as hTp, \
             tc.tile_pool(name="zt", bufs=3) as ztp, \
             tc.tile_pool(name="outp", bufs=4) as outp:

            mu2, rstd2 = layernorm_stats(hown, "2", bcD, sqD)

            # h2 = bf16( LN2(hblk)*w2 )
            h2 = []
            for dt in range(DT):
                z = ztp.tile([P, TOWN], F32, tag="zt")
                nc.vector.tensor_sub(z, hown[dt], mu2)
                nc.vector.tensor_mul(z, z, rstd2)
                h2t = h2p.tile([P, TOWN], BF16, name=f"h2_{dt}")
                nc.scalar.activation(h2t, z, AF.Copy,
                                     scale=w2_sb[:, dt:dt + 1])
                h2.append(h2t)

            hbT = [hTp.tile([P, D], F32, name=f"hbT_{ts}") for ts in range(TS)]
            ghid = ghidp.tile([P, HT, TOWN], BF16, name="ghid")

            with tc.tile_pool(name="psum_fc", bufs=2, space="PSUM") as psum_fc, \
                 tc.tile_pool(name="psum_tr", bufs=2, space="PSUM") as psum_tr:
                # transpose hblk (owned part) -> hbT[ts]: [128(t), 1024(d)]
                for dt in range(DT):
                    for ts in range(TS):
                        pt = psum_tr.tile([P, P], F32, tag="ptr")
                        nc.tensor.transpose(
                            pt, hown[dt][:, ts * P:(ts + 1) * P], ident)
                        nc.scalar.copy(hbT[ts][:, dt * P:(dt + 1) * P], pt)

                # fc + gelu: ghid[:, ht, :] = gelu(WfcT.T @ h2)   [hid | t]
                for ht in range(HT):
                    wfc = wfcp.tile([P, DT, P], BF16, tag="wfc")
                    nc.sync.dma_start(wfc, wfc_t[ht])
                    ps = psum_fc.tile([P, TOWN], F32, tag="psfc")
                    for dt in range(DT):
                        nc.tensor.matmul(ps, wfc[:, dt, :], h2[dt],
                                         start=(dt == 0), stop=(dt == DT - 1))
                    nc.scalar.activation(ghid[:, ht, :], ps,
                                         AF.Gelu_apprx_tanh)

            # proj: out[ts] = hbT[ts] + ghid.T @ WprojT   [t | d]
            with tc.tile_pool(name="psum_pj", bufs=1, space="PSUM") as psum_pj:
                pst = [[psum_pj.tile([P, TOWN], F32, name=f"pj_{ts}_{fs}")
                        for fs in range(2)] for ts in range(TS)]
                for hk in range(HT):
                    wpr = wprp.tile([P, D], BF16, tag="wpr")
                    nc.sync.dma_start(wpr, wprojT[hk * P:(hk + 1) * P, :])
                    for ts in range(TS):
                        for fs in range(2):
                            nc.tensor.matmul(
                                pst[ts][fs],
                                ghid[:, hk, ts * P:(ts + 1) * P],
                                wpr[:, fs * TOWN:(fs + 1) * TOWN],
                                start=(hk == 0), stop=(hk == HT - 1))
                for ts in range(TS):
                    ot = outp.tile([P, D], F32, tag="outt")
                    for fs in range(2):
                        nc.vector.tensor_add(
                            ot[:, fs * TOWN:(fs + 1) * TOWN], pst[ts][fs],
                            hbT[ts][:, fs * TOWN:(fs + 1) * TOWN])
                    nc.sync.dma_start(out_d[ts * P:(ts + 1) * P, :], ot)

    nc.to_json_bytes = types.MethodType(_patched_to_json_bytes, nc)
    return nc


# =====================================================================
# Host side
# =====================================================================
_CACHED = {}


def _get_nc():
    if "nc" not in _CACHED:
        _CACHED["nc"] = build_bass()
    return _CACHED["nc"]


def kernel(x, ln1_w, ln2_w, W_dbc, W_dt, b_dt, A_log, Dp, W_fc, W_proj):
    x = np.asarray(x, np.float32)
    f32 = lambda a: np.ascontiguousarray(np.asarray(a, np.float32))
    bf16 = lambda a: np.ascontiguousarray(
        np.asarray(a, np.float32).astype(ml_dtypes.bfloat16))

    wdbc = np.asarray(W_dbc, np.float32)                    # [96, D]
    wdbc_pad = np.zeros((P, D), np.float32)
    wdbc_pad[0:R] = wdbc[0:R]                # delta rows at 0
    wdbc_pad[64:64 + N] = wdbc[R:R + N]      # B rows at 64
    wdbc_pad[96:96 + N] = wdbc[R + N:]       # C rows at 96
    wdbcT = f32(wdbc_pad.T)                                 # [D, 128]
    wdtT = f32(np.asarray(W_dt, np.float32).T)              # [R, D]
    bdt_r = f32(np.asarray(b_dt, np.float32).reshape(DT, P).T)     # [P, DT]
    aneg = -np.exp(np.asarray(A_log, np.float32))           # [D, N]
    aneg_r = f32(aneg.reshape(DT, P, N).transpose(1, 0, 2))  # [P, DT, N]
    dp1_r = f32((np.asarray(Dp, np.float32) + 1.0).reshape(DT, P).T)
    w1_r = f32(np.asarray(ln1_w, np.float32).reshape(DT, P).T)
    w2_r = f32(np.asarray(ln2_w, np.float32).reshape(DT, P).T)
    # wfc_t[ht, p, dt, c] = W_fc[ht*P+c, dt*P+p]
    wfc4 = np.asarray(W_fc, np.float32).reshape(HT, P, DT, P)  # [ht,c,dt,p]
    wfc_t = bf16(wfc4.transpose(0, 3, 2, 1))                   # [ht,p,dt,c]
    wprojT = bf16(np.asarray(W_proj, np.float32).T)            # [HID, D]

    mask_on = np.ones((P, T), np.float32)
    mask_off = mask_on.copy()
    mask_off[:, :HALO] = 0.0

    in_maps = []
    for core in range(NCORES):
        b, half = core // 2, core % 2
        l0 = half * TOWN
        xb = x[b].T  # [D, L] feature-major
        if half == 0:
            x_fm = np.zeros((D, T), np.float32)
            x_fm[:, HALO:] = xb[:, :TOWN]
            msk = mask_off
        else:
            x_fm = np.ascontiguousarray(xb[:, l0 - HALO:l0 + TOWN])
            msk = mask_on
        in_maps.append({
            "x_fm": np.ascontiguousarray(x_fm), "mask": msk,
            "wdbcT": wdbcT, "wdtT": wdtT, "bdt_r": bdt_r,
            "aneg_r": aneg_r, "dp1_r": dp1_r, "w1_r": w1_r, "w2_r": w2_r,
            "wfc_t": wfc_t, "wprojT": wprojT,
        })

    res = run_bass_kernel_spmd(_get_nc(), in_maps, core_ids=list(range(NCORES)))
    _CACHED["last_res"] = res
    out = np.empty((B, L, D), np.float32)
    for core in range(NCORES):
        b, half = core // 2, core % 2
        out[b, half * TOWN:(half + 1) * TOWN, :] = res.results[core]["out"]
    return out


if __name__ == "__main__":
    nc = build_bass()
    print("build ok; instructions:",
          sum(1 for _ in nc.m.functions[0].instructions)
          if hasattr(nc.m.functions[0], "instructions") else "?")
